# revision 22
# baseline (speedup 1.0000x reference)
"""Batch-hard triplet loss on 8 Trainium2 NeuronCores.

Math (matches the reference exactly up to fp rounding):
  d_ij   = ||h_i||^2 + ||h_j||^2 - 2 h_i.h_j, clamped to [EPS, inf)
  hp_i   = max over j (same label, j != i) of d_ij
  hn_i   = 2nd-smallest over j (different label) of d_ij
  loss_i = max(hp_i - hn_i + ALPHA, 0)
  out    = sum(loss_i[loss_i > EPS]) / count(loss_i > EPS)

Device strategy: rows are sharded over 8 cores (1024 each). Each core runs
one augmented GEMM whose PSUM output is directly the mining quantity

  p_ij = 2 h_i.h_j - ||h_j||^2 - BIG * [label_i == label_j]

built from a K = D + C + (norm rows) contraction:
  a_i = ( 2 h_i,  -BIG * onehot(label_i),  ones )
  b_j = ( h_j,     onehot(label_j),        xnorm split rows )

Row-constant terms (||h_i||^2, the EPS clamp) cancel in hp - hn, so they are
never computed.  With t_ij := d_ij - ||h_i||^2 = -p_ij - BIG*eq:
  hp_i = -min_j(p_ij) - BIG        (positives carry -BIG, dominate the min;
                                    Sterbenz: the BIG subtraction is exact)
  hn_i = -max8(p_i)[1]             (negatives are the largest p; the DVE Max8
                                    instruction gives the top-8 descending, so
                                    element 1 is the 2nd-smallest distance,
                                    with tie multiplicity matching top_k)
  loss_i = max( max8[1] - min + (ALPHA - BIG), 0 )   (clamp applied on host)

Operands are packed on the host into [128, n_chunks, cols] chunk tensors so
each SBUF load is a single batched DMA.  dtype mode:
  "bf16": 10 K=128 bf16 matmuls per PSUM tile.
  "fp8":  h rounded to e4m3; 4 DoubleRow e4m3 matmuls (K=256 each) + 1
          DoubleRow e5m2 matmul carrying the one-hot mask (values -BIG/0/1,
          exact in e5m2) and ||h_j||^2 as a 6-term e5m2 expansion. 5 matmuls
          per tile, ~2x fewer PE cycles than bf16.

The masked mean over all 8192 rows is done on the host from the returned
per-row loss vectors (8 x 1024 floats).
"""

import functools

import numpy as np
import ml_dtypes

import concourse.bacc as bacc
import concourse.tile as tile
from concourse import mybir
from concourse.bass_utils import run_bass_kernel_spmd

BF16 = mybir.dt.bfloat16
FP8E4 = mybir.dt.float8e4
FP8E5 = mybir.dt.float8e5
F32 = mybir.dt.float32
E4 = ml_dtypes.float8_e4m3
E5 = ml_dtypes.float8_e5m2
BF = ml_dtypes.bfloat16

N, D, C = 8192, 1024, 128
NCORES = 8
P = 128
JB = 512  # matmul moving free dim = one fp32 PSUM bank
ALPHA = 0.1
EPS = 1e-7
BIG = 8192.0
NNORM = 6  # e5m2 expansion terms for ||h_j||^2 in fp8 mode
MODE = "fp8"  # "bf16" or "fp8"


def build_program(rows, n, d, c, jb, mode=MODE, psum_bufs=7, b_bufs=4):
    """Emit the per-core Bass/Tile program (identical on all cores)."""
    kh = d // P
    m_chunks = rows // P
    nj = n // jb
    assert rows % P == 0 and d % P == 0 and n % jb == 0 and c <= P

    nc = bacc.Bacc("TRN2", target_bir_lowering=False)
    if mode == "bf16":
        kc_tot = kh + 2
        A4 = nc.dram_tensor("A4", [P, kc_tot, rows], BF16, kind="ExternalInput")
        B4 = nc.dram_tensor("B4", [P, kc_tot, n], BF16, kind="ExternalInput")
        A5 = B5 = None
    else:
        assert kh % 2 == 0
        A4 = nc.dram_tensor("A4", [P, kh, rows], FP8E4, kind="ExternalInput")
        B4 = nc.dram_tensor("B4", [P, kh, n], FP8E4, kind="ExternalInput")
        A5 = nc.dram_tensor("A5", [P, 2, rows], FP8E5, kind="ExternalInput")
        B5 = nc.dram_tensor("B5", [P, 2, n], FP8E5, kind="ExternalInput")
    EYE = nc.dram_tensor("EYE", [P, P], F32, kind="ExternalInput")
    loss = nc.dram_tensor("loss", [rows], F32, kind="ExternalOutput")

    with tile.TileContext(nc) as tc:
        with (
            tc.tile_pool(name="apool", bufs=1) as apool,
            tc.tile_pool(name="bpool", bufs=b_bufs) as bpool,
            tc.tile_pool(name="psum", bufs=psum_bufs, space="PSUM") as pp,
            tc.tile_pool(name="pst", bufs=1, space="PSUM") as pst,
            tc.tile_pool(name="mpool", bufs=1) as mpool,
            tc.tile_pool(name="fpool", bufs=6) as fpool,
        ):
            # Warm the PE HAM clock gate while the first DMAs land: dummy
            # matmuls on a zeroed tile keep the PE busy through its
            # 4096-cycle activity window so real matmuls run at 2.4 GHz.
            wsrc = apool.tile([1, 16 + jb], BF16, tag="wsrc")
            nc.vector.memset(wsrc[:], 0.0)
            wps = pp.tile([P, jb], F32, name="ps", tag="ps")
            for _ in range(16):
                nc.tensor.matmul(wps[:16, :], wsrc[:1, :16], wsrc[:1, 16:],
                                 start=True, stop=True)

            def load_b(j):
                js = slice(j * jb, (j + 1) * jb)
                if mode == "bf16":
                    b4 = bpool.tile([P, kh + 2, jb], BF16, tag="b4", name="b4")
                    nc.sync.dma_start(out=b4[:], in_=B4[:, :, js])
                    return (b4, None)
                b4 = bpool.tile([P, kh, jb], FP8E4, tag="b4", name="b4")
                nc.sync.dma_start(out=b4[:], in_=B4[:, :, js])
                b5 = bpool.tile([P, 2, jb], FP8E5, tag="b5", name="b5")
                nc.sync.dma_start(out=b5[:], in_=B5[:, :, js])
                return (b4, b5)

            # First moving block before the stationary A so compute can
            # start as soon as possible.
            b_pre = load_b(0)

            # Stationary A, resident all kernel, loaded as one DMA per row
            # chunk so chunk m's operands land just before the PE needs
            # them (one big transfer stalled the pipeline and re-tripped
            # the HAM throttle).
            a4dt = BF16 if mode == "bf16" else FP8E4
            a4ks = kh + 2 if mode == "bf16" else kh
            a4m = []
            a5 = None
            b_pre2 = None
            b_pre3 = None
            for m in range(m_chunks):
                ms = slice(m * P, (m + 1) * P)
                t = apool.tile([P, a4ks, P], a4dt, tag=f"a4m{m}",
                               name=f"a4m{m}")
                nc.sync.dma_start(out=t[:], in_=A4[:, :, ms])
                a4m.append(t)
                if m == 0 and mode == "fp8":
                    a5 = apool.tile([P, 2, rows], FP8E5, tag="a5", name="a5")
                    nc.sync.dma_start(out=a5[:], in_=A5[:])
                if m == 2 and nj > 1:
                    # Prefetch the second moving block ahead of the
                    # remaining stationary chunks so j=1 never stalls.
                    b_pre2 = load_b(1)
                if m == 4 and nj > 2:
                    b_pre3 = load_b(2)
            eye = apool.tile([P, P], F32, tag="eye")
            nc.sync.dma_start(out=eye[:], in_=EYE[:])

            # Per-row-chunk partial mining results, merged after the j loop.
            # Host-side column rotation guarantees every own-class (positive)
            # column of this core's rows lives in the first `hpj` j-blocks,
            # so the hardest-positive min only scans those.
            hpj = min(nj, 3)
            v8 = [mpool.tile([P, nj * 8], F32, tag=f"v8_{m}", name=f"v8_{m}")
                  for m in range(m_chunks)]
            gmin = [mpool.tile([P, hpj], F32, tag=f"gm_{m}", name=f"gmin_{m}")
                    for m in range(m_chunks)]

            stage8 = mpool.tile([P, m_chunks], F32, tag="stage8")

            for j in range(nj):
                if j == 0:
                    b4, b5 = b_pre
                elif j == 1 and b_pre2 is not None:
                    b4, b5 = b_pre2
                elif j == 2 and b_pre3 is not None:
                    b4, b5 = b_pre3
                else:
                    b4, b5 = load_b(j)

                for m in range(m_chunks):
                    ps = pp.tile([P, jb], F32, name="ps", tag="ps")
                    at = a4m[m]
                    if mode == "bf16":
                        for kc in range(kh + 2):
                            nc.tensor.matmul(ps[:], at[:, kc, :],
                                             b4[:, kc, :],
                                             start=(kc == 0),
                                             stop=(kc == kh + 1))
                    else:
                        for t in range(kh // 2):
                            nc.tensor.matmul(
                                ps[:], at[:, 2 * t:2 * t + 2, :],
                                b4[:, 2 * t:2 * t + 2, :],
                                start=(t == 0), stop=False,
                                perf_mode=mybir.MatmulPerfMode.DoubleRow)
                        nc.tensor.matmul(
                            ps[:], a5[:, :, m * P:(m + 1) * P], b5[:],
                            start=False, stop=True,
                            perf_mode=mybir.MatmulPerfMode.DoubleRow)

                    nc.vector.max(v8[m][:, j * 8:(j + 1) * 8], ps[:])
                    if j < hpj:
                        nc.vector.tensor_reduce(gmin[m][:, j:j + 1], ps[:],
                                                axis=mybir.AxisListType.X,
                                                op=mybir.AluOpType.min)

                    if j == nj - 1:
                        # Final merge for this row chunk, interleaved so it
                        # overlaps the remaining row chunks' matmuls.
                        vf = fpool.tile([P, 8], F32, tag="vf", name="vf")
                        nc.vector.max(vf[:], v8[m][:])
                        gm = fpool.tile([P, 1], F32, tag="gm", name="gm")
                        nc.vector.tensor_reduce(gm[:], gmin[m][:],
                                                axis=mybir.AxisListType.X,
                                                op=mybir.AluOpType.min)
                        # loss_pre = (v2 + (ALPHA - BIG)) - gmin
                        nc.vector.scalar_tensor_tensor(
                            out=stage8[:, m:m + 1], in0=vf[:, 1:2],
                            scalar=float(ALPHA - BIG), in1=gm[:],
                            op0=mybir.AluOpType.add,
                            op1=mybir.AluOpType.subtract)

            # One transpose -> [m_chunks, 128] so the output DMA is a few
            # contiguous segments instead of 128 scattered 4B writes.
            pt = pst.tile([m_chunks, P], F32, name="pt")
            nc.tensor.transpose(pt[:], stage8[:], eye[:])
            outt = fpool.tile([m_chunks, P], F32, tag="outt", name="outt")
            nc.scalar.copy(outt[:], pt[:])
            nc.sync.dma_start(
                out=loss[:].rearrange("(m p) -> m p", p=P), in_=outt[:])

    nc.compile()
    return nc


def _split_e5(x, terms):
    """Greedy e5m2 expansion: x ~ sum of `terms` e5m2 rows (f64 in/out)."""
    out = []
    r = x.astype(np.float64).copy()
    for _ in range(terms):
        s = r.astype(E5)
        out.append(s)
        r -= s.astype(np.float64)
    return out


def make_inputs(H, labels, n, d, c, ncores, mode=MODE):
    """Host-side packing of the augmented GEMM operands.

    Rows are sorted by label and sharded contiguously.  Each core's B
    columns are rotated so every column whose label appears among that
    core's rows sits in the leading block (always < 3 * JB columns), which
    lets the device mine the hardest positive from the first 3 j-blocks
    only.  The final masked mean is permutation invariant, so neither the
    sort nor the rotations need undoing.
    """
    H = np.ascontiguousarray(np.asarray(H, dtype=np.float32))
    labels = np.asarray(labels).astype(np.int64).ravel()
    kh = d // P
    rows = n // ncores

    perm = np.argsort(labels, kind="stable")
    H = H[perm]
    labels = labels[perm]
    col_orders = []
    for cix in range(ncores):
        own = np.zeros(n, dtype=bool)
        own[np.isin(labels, labels[cix * rows:(cix + 1) * rows])] = True
        order = np.concatenate([np.nonzero(own)[0], np.nonzero(~own)[0]])
        assert own.sum() <= min(n, 3 * JB), own.sum()
        col_orders.append(order)

    oh = labels[None, :] == np.arange(c, dtype=np.int64)[:, None]  # [c, n]
    EYEM = np.eye(P, dtype=np.float32)

    if mode == "bf16":
        Hr = H.astype(BF)
        Hr32 = Hr.astype(np.float32)
        xn = np.einsum("ij,ij->i", Hr.astype(np.float64), Hr.astype(np.float64))
        xh = xn.astype(BF)
        xl = (xn - xh.astype(np.float64)).astype(BF)

        B4m = np.zeros((P, kh + 2, n), dtype=BF)
        B4m[:, :kh, :] = Hr.T.reshape(kh, P, n).transpose(1, 0, 2)
        B4m[:c, kh, :] = oh.astype(BF)
        B4m[0, kh + 1, :] = xh
        B4m[1, kh + 1, :] = xl

        in_maps = []
        for cix in range(ncores):
            sl = slice(cix * rows, (cix + 1) * rows)
            A4m = np.zeros((P, kh + 2, rows), dtype=BF)
            A4m[:, :kh, :] = ((2.0 * Hr32[sl].T).astype(BF)
                              .reshape(kh, P, rows).transpose(1, 0, 2))
            A4m[:c, kh, :] = (-BIG * oh[:, sl]).astype(BF)
            A4m[0:2, kh + 1, :] = -1.0
            in_maps.append({"A4": A4m, "B4": B4m[:, :, col_orders[cix]],
                            "EYE": EYEM})
        return in_maps

    # fp8 mode
    Hr = H.astype(E4)
    Hr32 = Hr.astype(np.float32)
    xn = np.einsum("ij,ij->i", Hr.astype(np.float64), Hr.astype(np.float64))
    xsplit = _split_e5(xn, NNORM)

    B4m = Hr.T.reshape(kh, P, n).transpose(1, 0, 2).copy()  # [P, kh, n] e4m3
    B5m = np.zeros((P, 2, n), dtype=E5)
    B5m[:c, 0, :] = oh.astype(E5)
    for t in range(NNORM):
        B5m[t, 1, :] = xsplit[t]

    in_maps = []
    for cix in range(ncores):
        sl = slice(cix * rows, (cix + 1) * rows)
        A4m = ((2.0 * Hr32[sl].T).astype(E4)
               .reshape(kh, P, rows).transpose(1, 0, 2).copy())
        A5m = np.zeros((P, 2, rows), dtype=E5)
        A5m[:c, 0, :] = (-BIG * oh[:, sl]).astype(E5)
        A5m[:NNORM, 1, :] = -1.0
        in_maps.append({"A4": A4m, "B4": B4m[:, :, col_orders[cix]],
                        "A5": A5m, "B5": B5m[:, :, col_orders[cix]],
                        "EYE": EYEM})
    return in_maps


@functools.lru_cache(maxsize=2)
def _get_program(mode=MODE):
    return build_program(N // NCORES, N, D, C, JB, mode=mode)


def _finalize(loss_rows):
    loss_all = np.concatenate(
        [np.asarray(l, dtype=np.float64) for l in loss_rows])
    loss_all = np.maximum(loss_all, 0.0)
    rel = loss_all > EPS
    cnt = int(rel.sum())
    if cnt == 0:
        return np.float32(0.0)
    return np.float32(loss_all[rel].sum() / cnt)


def kernel(H, labels):
    in_maps = make_inputs(H, labels, N, D, C, NCORES)
    res = run_bass_kernel_spmd(_get_program(), in_maps, list(range(NCORES)))
    return _finalize([r["loss"] for r in res.results])


# revision 23
# speedup vs baseline: 1.0153x; 1.0153x over previous
"""Batch-hard triplet loss on 8 Trainium2 NeuronCores.

Math (matches the reference exactly up to fp rounding):
  d_ij   = ||h_i||^2 + ||h_j||^2 - 2 h_i.h_j, clamped to [EPS, inf)
  hp_i   = max over j (same label, j != i) of d_ij
  hn_i   = 2nd-smallest over j (different label) of d_ij
  loss_i = max(hp_i - hn_i + ALPHA, 0)
  out    = sum(loss_i[loss_i > EPS]) / count(loss_i > EPS)

Device strategy: rows are sharded over 8 cores (1024 each). Each core runs
one augmented GEMM whose PSUM output is directly the mining quantity

  p_ij = 2 h_i.h_j - ||h_j||^2 - BIG * [label_i == label_j]

built from a K = D + C + (norm rows) contraction:
  a_i = ( 2 h_i,  -BIG * onehot(label_i),  ones )
  b_j = ( h_j,     onehot(label_j),        xnorm split rows )

Row-constant terms (||h_i||^2, the EPS clamp) cancel in hp - hn, so they are
never computed.  With t_ij := d_ij - ||h_i||^2 = -p_ij - BIG*eq:
  hp_i = -min_j(p_ij) - BIG        (positives carry -BIG, dominate the min;
                                    Sterbenz: the BIG subtraction is exact)
  hn_i = -max8(p_i)[1]             (negatives are the largest p; the DVE Max8
                                    instruction gives the top-8 descending, so
                                    element 1 is the 2nd-smallest distance,
                                    with tie multiplicity matching top_k)
  loss_i = max( max8[1] - min + (ALPHA - BIG), 0 )   (clamp applied on host)

Operands are packed on the host into [128, n_chunks, cols] chunk tensors so
each SBUF load is a single batched DMA.  dtype mode:
  "bf16": 10 K=128 bf16 matmuls per PSUM tile.
  "fp8":  h rounded to e4m3; 4 DoubleRow e4m3 matmuls (K=256 each) + 1
          DoubleRow e5m2 matmul carrying the one-hot mask (values -BIG/0/1,
          exact in e5m2) and ||h_j||^2 as a 6-term e5m2 expansion. 5 matmuls
          per tile, ~2x fewer PE cycles than bf16.

The masked mean over all 8192 rows is done on the host from the returned
per-row loss vectors (8 x 1024 floats).
"""

import functools

import numpy as np
import ml_dtypes

import concourse.bacc as bacc
import concourse.tile as tile
from concourse import mybir
from concourse.bass_utils import run_bass_kernel_spmd

BF16 = mybir.dt.bfloat16
FP8E4 = mybir.dt.float8e4
FP8E5 = mybir.dt.float8e5
F32 = mybir.dt.float32
E4 = ml_dtypes.float8_e4m3
E5 = ml_dtypes.float8_e5m2
BF = ml_dtypes.bfloat16

N, D, C = 8192, 1024, 128
NCORES = 8
P = 128
JB = 512  # matmul moving free dim = one fp32 PSUM bank
ALPHA = 0.1
EPS = 1e-7
BIG = 8192.0
NNORM = 6  # e5m2 expansion terms for ||h_j||^2 in fp8 mode
MODE = "fp8"  # "bf16" or "fp8"


def build_program(rows, n, d, c, jb, mode=MODE, psum_bufs=7, b_bufs=3):
    """Emit the per-core Bass/Tile program (identical on all cores)."""
    kh = d // P
    m_chunks = rows // P
    nj = n // jb
    assert rows % P == 0 and d % P == 0 and n % jb == 0 and c <= P

    nc = bacc.Bacc("TRN2", target_bir_lowering=False)
    if mode == "bf16":
        kc_tot = kh + 2
        A4 = nc.dram_tensor("A4", [P, kc_tot, rows], BF16, kind="ExternalInput")
        B4 = nc.dram_tensor("B4", [P, kc_tot, n], BF16, kind="ExternalInput")
        A5 = B5 = None
    else:
        assert kh % 2 == 0
        A4 = nc.dram_tensor("A4", [P, kh, rows], FP8E4, kind="ExternalInput")
        B4 = nc.dram_tensor("B4", [P, kh, n], FP8E4, kind="ExternalInput")
        A5 = nc.dram_tensor("A5", [P, 2, rows], FP8E5, kind="ExternalInput")
        B5 = nc.dram_tensor("B5", [P, 2, n], FP8E5, kind="ExternalInput")
    EYE = nc.dram_tensor("EYE", [P, P], F32, kind="ExternalInput")
    loss = nc.dram_tensor("loss", [rows], F32, kind="ExternalOutput")

    with tile.TileContext(nc) as tc:
        with (
            tc.tile_pool(name="apool", bufs=1) as apool,
            tc.tile_pool(name="bpool", bufs=b_bufs) as bpool,
            tc.tile_pool(name="psum", bufs=psum_bufs, space="PSUM") as pp,
            tc.tile_pool(name="pst", bufs=1, space="PSUM") as pst,
            tc.tile_pool(name="mpool", bufs=1) as mpool,
            tc.tile_pool(name="fpool", bufs=6) as fpool,
        ):
            # Warm the PE HAM clock gate while the first DMAs land: dummy
            # matmuls on a zeroed tile keep the PE busy through its
            # 4096-cycle activity window so real matmuls run at 2.4 GHz.
            wsrc = apool.tile([1, 16 + jb], BF16, tag="wsrc")
            nc.vector.memset(wsrc[:], 0.0)
            wps = pp.tile([P, jb], F32, name="ps", tag="ps")
            for _ in range(8):
                nc.tensor.matmul(wps[:16, :], wsrc[:1, :16], wsrc[:1, 16:],
                                 start=True, stop=True)

            def load_b(j):
                js = slice(j * jb, (j + 1) * jb)
                if mode == "bf16":
                    b4 = bpool.tile([P, kh + 2, jb], BF16, tag="b4", name="b4")
                    nc.sync.dma_start(out=b4[:], in_=B4[:, :, js])
                    return (b4, None)
                b4 = bpool.tile([P, kh, jb], FP8E4, tag="b4", name="b4")
                nc.sync.dma_start(out=b4[:], in_=B4[:, :, js])
                b5 = bpool.tile([P, 2, jb], FP8E5, tag="b5", name="b5")
                nc.sync.dma_start(out=b5[:], in_=B5[:, :, js])
                return (b4, b5)

            # First moving block before the stationary A so compute can
            # start as soon as possible.
            b_pre = load_b(0)

            # Stationary A, resident all kernel, loaded as one DMA per row
            # chunk so chunk m's operands land just before the PE needs
            # them (one big transfer stalled the pipeline and re-tripped
            # the HAM throttle).
            a4dt = BF16 if mode == "bf16" else FP8E4
            a4ks = kh + 2 if mode == "bf16" else kh
            a4m = []
            a5 = None
            b_pre2 = None
            for m in range(m_chunks):
                ms = slice(m * P, (m + 1) * P)
                t = apool.tile([P, a4ks, P], a4dt, tag=f"a4m{m}",
                               name=f"a4m{m}")
                nc.sync.dma_start(out=t[:], in_=A4[:, :, ms])
                a4m.append(t)
                if m == 0 and mode == "fp8":
                    a5 = apool.tile([P, 2, rows], FP8E5, tag="a5", name="a5")
                    nc.sync.dma_start(out=a5[:], in_=A5[:])
                if m == 2 and nj > 1:
                    # Prefetch the second moving block ahead of the
                    # remaining stationary chunks so j=1 never stalls.
                    b_pre2 = load_b(1)
            eye = apool.tile([P, P], F32, tag="eye")
            nc.sync.dma_start(out=eye[:], in_=EYE[:])

            # Per-row-chunk partial mining results, merged after the j loop.
            # Host-side column rotation guarantees every own-class (positive)
            # column of this core's rows lives in the first `hpj` j-blocks,
            # so the hardest-positive min only scans those.
            hpj = min(nj, 3)
            v8 = [mpool.tile([P, nj * 8], F32, tag=f"v8_{m}", name=f"v8_{m}")
                  for m in range(m_chunks)]
            gmin = [mpool.tile([P, hpj], F32, tag=f"gm_{m}", name=f"gmin_{m}")
                    for m in range(m_chunks)]

            stage8 = mpool.tile([P, m_chunks], F32, tag="stage8")

            for j in range(nj):
                if j == 0:
                    b4, b5 = b_pre
                elif j == 1 and b_pre2 is not None:
                    b4, b5 = b_pre2
                else:
                    b4, b5 = load_b(j)

                for m in range(m_chunks):
                    ps = pp.tile([P, jb], F32, name="ps", tag="ps")
                    at = a4m[m]
                    if mode == "bf16":
                        for kc in range(kh + 2):
                            nc.tensor.matmul(ps[:], at[:, kc, :],
                                             b4[:, kc, :],
                                             start=(kc == 0),
                                             stop=(kc == kh + 1))
                    else:
                        for t in range(kh // 2):
                            nc.tensor.matmul(
                                ps[:], at[:, 2 * t:2 * t + 2, :],
                                b4[:, 2 * t:2 * t + 2, :],
                                start=(t == 0), stop=False,
                                perf_mode=mybir.MatmulPerfMode.DoubleRow)
                        nc.tensor.matmul(
                            ps[:], a5[:, :, m * P:(m + 1) * P], b5[:],
                            start=False, stop=True,
                            perf_mode=mybir.MatmulPerfMode.DoubleRow)

                    nc.vector.max(v8[m][:, j * 8:(j + 1) * 8], ps[:])
                    if j < hpj:
                        nc.vector.tensor_reduce(gmin[m][:, j:j + 1], ps[:],
                                                axis=mybir.AxisListType.X,
                                                op=mybir.AluOpType.min)

                    if j == nj - 1:
                        # Final merge for this row chunk, interleaved so it
                        # overlaps the remaining row chunks' matmuls.
                        vf = fpool.tile([P, 8], F32, tag="vf", name="vf")
                        nc.vector.max(vf[:], v8[m][:])
                        gm = fpool.tile([P, 1], F32, tag="gm", name="gm")
                        nc.vector.tensor_reduce(gm[:], gmin[m][:],
                                                axis=mybir.AxisListType.X,
                                                op=mybir.AluOpType.min)
                        # loss_pre = (v2 + (ALPHA - BIG)) - gmin
                        nc.vector.scalar_tensor_tensor(
                            out=stage8[:, m:m + 1], in0=vf[:, 1:2],
                            scalar=float(ALPHA - BIG), in1=gm[:],
                            op0=mybir.AluOpType.add,
                            op1=mybir.AluOpType.subtract)

            # One transpose -> [m_chunks, 128] so the output DMA is a few
            # contiguous segments instead of 128 scattered 4B writes.
            pt = pst.tile([m_chunks, P], F32, name="pt")
            nc.tensor.transpose(pt[:], stage8[:], eye[:])
            outt = fpool.tile([m_chunks, P], F32, tag="outt", name="outt")
            nc.scalar.copy(outt[:], pt[:])
            nc.sync.dma_start(
                out=loss[:].rearrange("(m p) -> m p", p=P), in_=outt[:])

    nc.compile()
    return nc


def _split_e5(x, terms):
    """Greedy e5m2 expansion: x ~ sum of `terms` e5m2 rows (f64 in/out)."""
    out = []
    r = x.astype(np.float64).copy()
    for _ in range(terms):
        s = r.astype(E5)
        out.append(s)
        r -= s.astype(np.float64)
    return out


def make_inputs(H, labels, n, d, c, ncores, mode=MODE):
    """Host-side packing of the augmented GEMM operands.

    Rows are sorted by label and sharded contiguously.  Each core's B
    columns are rotated so every column whose label appears among that
    core's rows sits in the leading block (always < 3 * JB columns), which
    lets the device mine the hardest positive from the first 3 j-blocks
    only.  The final masked mean is permutation invariant, so neither the
    sort nor the rotations need undoing.
    """
    H = np.ascontiguousarray(np.asarray(H, dtype=np.float32))
    labels = np.asarray(labels).astype(np.int64).ravel()
    kh = d // P
    rows = n // ncores

    perm = np.argsort(labels, kind="stable")
    H = H[perm]
    labels = labels[perm]
    col_orders = []
    for cix in range(ncores):
        own = np.zeros(n, dtype=bool)
        own[np.isin(labels, labels[cix * rows:(cix + 1) * rows])] = True
        order = np.concatenate([np.nonzero(own)[0], np.nonzero(~own)[0]])
        assert own.sum() <= min(n, 3 * JB), own.sum()
        col_orders.append(order)

    oh = labels[None, :] == np.arange(c, dtype=np.int64)[:, None]  # [c, n]
    EYEM = np.eye(P, dtype=np.float32)

    if mode == "bf16":
        Hr = H.astype(BF)
        Hr32 = Hr.astype(np.float32)
        xn = np.einsum("ij,ij->i", Hr.astype(np.float64), Hr.astype(np.float64))
        xh = xn.astype(BF)
        xl = (xn - xh.astype(np.float64)).astype(BF)

        B4m = np.zeros((P, kh + 2, n), dtype=BF)
        B4m[:, :kh, :] = Hr.T.reshape(kh, P, n).transpose(1, 0, 2)
        B4m[:c, kh, :] = oh.astype(BF)
        B4m[0, kh + 1, :] = xh
        B4m[1, kh + 1, :] = xl

        in_maps = []
        for cix in range(ncores):
            sl = slice(cix * rows, (cix + 1) * rows)
            A4m = np.zeros((P, kh + 2, rows), dtype=BF)
            A4m[:, :kh, :] = ((2.0 * Hr32[sl].T).astype(BF)
                              .reshape(kh, P, rows).transpose(1, 0, 2))
            A4m[:c, kh, :] = (-BIG * oh[:, sl]).astype(BF)
            A4m[0:2, kh + 1, :] = -1.0
            in_maps.append({"A4": A4m, "B4": B4m[:, :, col_orders[cix]],
                            "EYE": EYEM})
        return in_maps

    # fp8 mode
    Hr = H.astype(E4)
    Hr32 = Hr.astype(np.float32)
    xn = np.einsum("ij,ij->i", Hr.astype(np.float64), Hr.astype(np.float64))
    xsplit = _split_e5(xn, NNORM)

    B4m = Hr.T.reshape(kh, P, n).transpose(1, 0, 2).copy()  # [P, kh, n] e4m3
    B5m = np.zeros((P, 2, n), dtype=E5)
    B5m[:c, 0, :] = oh.astype(E5)
    for t in range(NNORM):
        B5m[t, 1, :] = xsplit[t]

    in_maps = []
    for cix in range(ncores):
        sl = slice(cix * rows, (cix + 1) * rows)
        A4m = ((2.0 * Hr32[sl].T).astype(E4)
               .reshape(kh, P, rows).transpose(1, 0, 2).copy())
        A5m = np.zeros((P, 2, rows), dtype=E5)
        A5m[:c, 0, :] = (-BIG * oh[:, sl]).astype(E5)
        A5m[:NNORM, 1, :] = -1.0
        in_maps.append({"A4": A4m, "B4": B4m[:, :, col_orders[cix]],
                        "A5": A5m, "B5": B5m[:, :, col_orders[cix]],
                        "EYE": EYEM})
    return in_maps


@functools.lru_cache(maxsize=2)
def _get_program(mode=MODE):
    return build_program(N // NCORES, N, D, C, JB, mode=mode)


def _finalize(loss_rows):
    loss_all = np.concatenate(
        [np.asarray(l, dtype=np.float64) for l in loss_rows])
    loss_all = np.maximum(loss_all, 0.0)
    rel = loss_all > EPS
    cnt = int(rel.sum())
    if cnt == 0:
        return np.float32(0.0)
    return np.float32(loss_all[rel].sum() / cnt)


def kernel(H, labels):
    in_maps = make_inputs(H, labels, N, D, C, NCORES)
    res = run_bass_kernel_spmd(_get_program(), in_maps, list(range(NCORES)))
    return _finalize([r["loss"] for r in res.results])
